# revision 31
# baseline (speedup 1.0000x reference)
"""Trainium2 Bass kernel for nn_DegreePrediction (RBC via batched Perron vectors).

Math: M[s,t] = weights_r*r_zeros + r_const is positive column-stochastic
(columns sum to 1); its eigenvalue-1 right eigenvector is the Perron
vector and rbc[n] = sum_{s,t} T[s,t]/v[s,t,s] * v[s,t,n] is scale-free in
v.  v ~= M^2 @ ones to ~lambda2^2 ~ 0.4% << the 2e-2 gate, so two batched
mat-vec sweeps suffice (no squarings, no transposes).

Layout trick: each core's 512 matrices are uploaded TRANSPOSED in fp8-e3m4
(prescaled by 128 so entries sit in the e3m4 normal range; the result is
scale-free), two per 128-partition stack: MT[j+64h, 64q+i] ~ M_{2q+h}[i,j].
With lhsT = a [128,128] MT block (stationary operand) both sweeps keep
their results in the PARTITION dim:
  pass A: rhs = ones-blocks [128,2]       -> out[m,n] = rowsums w_p[m]
  pass B: rhs = block-diag w cols [128,4] -> out[m,n] = v_p[m]
fp8 LDWEIGHTS/MATMUL pairs pipeline through the PE reorder window at
~27ns/pair, so the kernel is paced by the 8-chunk 256KB DMA stream and
the cross-engine dependency hops.  The pipeline is split in column halves
so pass B of half 0 and its tail overlap the work of half 1.

Tail: the denominator row v_p[s_p] is gathered with a host mask (fp8,
issued as the 2nd DMA so it lands early) * VV in f32 on DVE, then an
fp32 ones-matmul partition-reduce -> d; 1/d = exp(-ln d) on ACT with both
functions forced into ONE act table (loaded once, in the DMA window);
coefficients T/d are broadcast down the partitions with a tiny PE matmul
and the final half-fold + column->row transpose is a single K=128 matmul
so the output DMA is one contiguous 256B descriptor.

Sharding: pairs split by s across 8 cores; host sums the partials.
"""

import numpy as np

_N = 64
_NCORES = 8
_NP = 512          # pairs per core
_NQ = 128          # double-stacks (4 pairs each)
_NCHUNK = 8        # DMA chunks of MT
_CW = _NQ * 128 // _NCHUNK   # MT cols per chunk (2048)
_QPC = _NQ // _NCHUNK        # double-stacks per chunk (16)
_MSCALE = 128.0    # power-of-2 prescale so fp8-e3m4 entries are normal

_cached = {}


def _build_program():
    import concourse.tile as tile
    from concourse import bacc, mybir
    from contextlib import ExitStack

    f32 = mybir.dt.float32
    bf16 = mybir.dt.bfloat16
    fp8 = mybir.dt.float8e3
    AF = mybir.ActivationFunctionType

    class _BaccOneActTable(bacc.Bacc):
        """Force Ln/Exp/Copy onto the single act-func set that contains all
        of them (natural_log_exp_and_others), so exactly one ACT_TABLE_LOAD
        is emitted (hoistable) instead of a Ln/Exp table ping-pong (3 extra
        1.5us loads in the tail).  Indices of the table list are preserved,
        only the contents of the other sets are blanked, so the
        act_func_set_id written into the BIR stays globally correct."""

        def insert_act_table_loads(self):
            import bass_rust as _bass_rust
            from concourse.hw_specs import get_activation_tables

            has_activation = any(
                isinstance(i, mybir.InstActivation)
                for b in self.main_func.blocks
                for i in b.instructions
            )
            if not has_activation:
                return
            tables = [
                (n, (f if n == "natural_log_exp_and_others" else set()))
                for n, f in get_activation_tables(self.m.arch).items()
            ]
            _bass_rust.insert_act_table_loads(self, tables)

    nc = _BaccOneActTable("TRN2", target_bir_lowering=False, debug=False)
    mt_in = nc.dram_tensor("mt", [_NCHUNK, 128, _CW], fp8,
                           kind="ExternalInput").ap()
    maskd_in = nc.dram_tensor("maskd", [128, _NP], fp8, kind="ExternalInput").ap()
    tm2_in = nc.dram_tensor("tm2", [128, _NP], f32, kind="ExternalInput").ap()
    out_dram = nc.dram_tensor("out", [128, 2], f32, kind="ExternalOutput").ap()

    with tile.TileContext(nc) as tc:
        with ExitStack() as ctx:
            consts = ctx.enter_context(tc.tile_pool(name="consts", bufs=1))
            psum = ctx.enter_context(tc.tile_pool(name="psum", bufs=1, space="PSUM"))

            # ---- stream MT chunks first (contiguous, in order); maskd is
            # the 2nd DMA so the half-0 dmm never waits on it; the other
            # small inputs trail the chunk stream ----
            mtc = []
            for d in range(_NCHUNK):
                t = consts.tile([128, _CW], fp8, tag=f"mt{d}")
                if d == 0:
                    # split the PE-gating first chunk and issue it from the
                    # ACT HWDGE ring: ACT reaches its first instruction
                    # ~0.5us before Sync clears its entry drain sequence
                    with tc.high_priority():
                        nc.scalar.dma_start(out=t[:, 0:_CW // 2],
                                            in_=mt_in[d, :, 0:_CW // 2])
                        nc.scalar.dma_start(out=t[:, _CW // 2:_CW],
                                            in_=mt_in[d, :, _CW // 2:_CW])
                else:
                    nc.sync.dma_start(out=t[:, :], in_=mt_in[d, :, :])
                mtc.append(t)
                if d == 0:
                    maskd = consts.tile([128, _NP], fp8)
                    nc.sync.dma_start(out=maskd[:, :], in_=maskd_in[:, :])

            tm2 = consts.tile([128, _NP], f32)
            nc.sync.dma_start(out=tm2[:, :], in_=tm2_in[:, :])

            ones2 = consts.tile([128, 2], bf16)
            ones128 = consts.tile([128, 1], bf16)
            nc.vector.memset(ones128[:, :], 1.0)
            nc.vector.memset(ones2[:, :], 0.0)
            nc.vector.memset(ones2[0:64, 0:1], 1.0)
            nc.vector.memset(ones2[64:128, 1:2], 1.0)
            one1 = consts.tile([1, 1], f32)
            nc.vector.memset(one1[:, :], 1.0)
            onesrow = consts.tile([1, 128], bf16)
            nc.vector.memset(onesrow[:, :], 1.0)
            L = consts.tile([128, 4 * _NQ], bf16)
            nc.vector.memset(L[:, :], 0.0)
            # hoist the single ln/exp ACT table load into the DMA window
            scratch = consts.tile([1, 1], f32)
            with tc.high_priority():
                nc.scalar.activation(out=scratch[:, :], in_=one1[:, :],
                                     func=AF.Ln)

            WW = psum.tile([128, 2 * _NQ], f32, tag="WW")
            VV = psum.tile([128, _NP], f32, tag="VV")
            DB = psum.tile([128, _NP], f32, tag="DB")
            WWv = WW[:, :].rearrange("p (q two) -> p q two", two=2)
            Lv = L[:, :].rearrange("p (q four) -> p q four", four=4)
            dmm = consts.tile([128, _NP], bf16)
            U = consts.tile([128, _NP], f32)
            DPS = psum.tile([1, _NP], f32, tag="DPS")
            dinv = consts.tile([1, _NP], f32)
            dinvb = consts.tile([1, _NP], bf16)
            vc = consts.tile([128, _NP], f32)
            r1 = consts.tile([128, 2], f32)

            def sweepA(h):
                for Q in range(64 * h, 64 * h + 64):
                    d, r = Q // _QPC, Q % _QPC
                    nc.tensor.matmul(
                        out=WW[:, 2 * Q:2 * Q + 2],
                        lhsT=mtc[d][:, 128 * r:128 * r + 128],
                        rhs=ones2[:, :], start=True, stop=True)

            def lbuild(h):
                qs = slice(64 * h, 64 * h + 64)
                nc.vector.tensor_copy(out=Lv[0:64, qs, 0], in_=WWv[0:64, qs, 0])
                nc.vector.tensor_copy(out=Lv[64:128, qs, 1], in_=WWv[0:64, qs, 1])
                nc.vector.tensor_copy(out=Lv[0:64, qs, 2], in_=WWv[64:128, qs, 0])
                nc.vector.tensor_copy(out=Lv[64:128, qs, 3], in_=WWv[64:128, qs, 1])

            def sweepB(h):
                for Q in range(64 * h, 64 * h + 64):
                    d, r = Q // _QPC, Q % _QPC
                    nc.tensor.matmul(
                        out=VV[:, 4 * Q:4 * Q + 4],
                        lhsT=mtc[d][:, 128 * r:128 * r + 128],
                        rhs=L[:, 4 * Q:4 * Q + 4], start=True, stop=True)

            def tail_dve(h):
                sl = slice(256 * h, 256 * h + 256)
                nc.vector.tensor_mul(out=dmm[:, sl], in0=VV[:, sl],
                                     in1=maskd[:, sl])

            def tail_d(h):
                sl = slice(256 * h, 256 * h + 256)
                # partition-reduce via ones-matmul (fp32 to keep d exact;
                # a GpSimd axis=C reduce is ~20x slower, measured)
                nc.tensor.matmul(out=DPS[:, sl], lhsT=ones128[:, :],
                                 rhs=dmm[:, sl], start=True, stop=True)

            def tail_u(h):
                sl = slice(256 * h, 256 * h + 256)
                # off the critical chain: T-weighting of v, no d dependency
                nc.vector.tensor_mul(out=U[:, sl], in0=VV[:, sl],
                                     in1=tm2[:, sl])

            def tail_coef(h):
                sl = slice(256 * h, 256 * h + 256)
                # 1/d = exp(-ln d): d > 0 (Perron); both funcs live in the
                # single preloaded table (ACT Reciprocal is banned for
                # accuracy, DVE divide on a 1-partition row is 8 cyc/elem)
                # ln in f32 (bf16 here would exponentiate into ~4% error);
                # the single bf16 rounding happens at the exp output
                nc.scalar.activation(out=dinv[:, sl], in_=DPS[:, sl],
                                     func=AF.Ln)
                nc.scalar.activation(out=dinvb[:, sl], in_=dinv[:, sl],
                                     func=AF.Exp, scale=-1.0)

            def tail_db(h):
                sl = slice(256 * h, 256 * h + 256)
                # broadcast dinv down all partitions: outer(ones, dinv)
                nc.tensor.matmul(out=DB[:, sl], lhsT=onesrow[:, :],
                                 rhs=dinvb[:, sl], start=True, stop=True)

            def tail_fin(h):
                sl = slice(256 * h, 256 * h + 256)
                nc.vector.tensor_mul(out=vc[:, sl], in0=U[:, sl],
                                     in1=DB[:, sl])
                nc.vector.tensor_reduce(
                    out=r1[:, h:h + 1], in_=vc[:, sl],
                    axis=mybir.AxisListType.X, op=mybir.AluOpType.add)

            # PE queue: A0 A1 B0 DPS0 B1 CB0 DPS1 CB1 FR.  Both A-sweeps
            # run first (purely DMA-paced; the lbuild casts overlap them on
            # DVE), so B0/B1 run back-to-back right after the last chunk
            # lands and the serial half-1 tail chain starts ~5us earlier,
            # with half-0's chain overlapping it on ACT/DVE.
            sweepA(0)
            lbuild(0)
            sweepA(1)
            lbuild(1)
            sweepB(0)
            tail_dve(0)
            tail_u(0)
            tail_d(0)
            sweepB(1)
            tail_coef(0)
            tail_dve(1)    # dmm(1)/U(1) precede fin(0) in the DVE queue:
            tail_u(1)      # they head the serial half-1 chain
            tail_db(0)
            tail_d(1)
            tail_coef(1)
            tail_fin(0)
            tail_db(1)
            tail_fin(1)

            # ship both half-reduces raw; the b-fold and node transpose
            # happen on host inside the existing cross-core gather
            nc.sync.dma_start(out=out_dram[:, :], in_=r1[:, :])
    nc.compile()
    return nc


def _get_program():
    if "nc" not in _cached:
        _cached["nc"] = _build_program()
    return _cached["nc"]


def _build_in_maps(x, weights_t, r_const):
    """Host-side layouts for all 8 cores."""
    import ml_dtypes

    M_all = r_const.reshape(_N * _N, _N, _N)
    i = np.arange(_N)
    r_diag = r_const[i[:, None], i[None, :], i[:, None], i[:, None]]
    T_full = (x * weights_t * r_diag).astype(np.float32)      # [64, 64]

    p = np.arange(_NP)
    b = (p >> 1) & 1                                          # stack-half of pair
    s_loc = p >> 6
    t_loc = p & 63

    in_maps = []
    for c in range(_NCORES):
        Mc = np.asarray(M_all[_NP * c:_NP * (c + 1)], np.float32)  # (p,i,j)
        # MT[j+64h, 64(2Q+b)+i] = scale*Mc[4Q+2b+h, i, j], chunked contiguously
        mt = ((Mc * _MSCALE).reshape(_NQ, 2, 2, _N, _N)  # (Q, b, h, i, j)
              .transpose(2, 4, 0, 1, 3)                  # (h, j, Q, b, i)
              .reshape(128, _NQ * 128))
        mt = (mt.reshape(128, _NCHUNK, _CW).transpose(1, 0, 2))  # (chunk, p, f)
        mt = np.ascontiguousarray(mt).astype(ml_dtypes.float8_e3m4)

        maskd = np.zeros((128, _NP), np.float32)
        maskd[64 * b + 8 * c + s_loc, p] = 1.0   # v's node index is GLOBAL s
        maskd = maskd.astype(ml_dtypes.float8_e3m4)

        Tp = T_full[8 * c + s_loc, t_loc]                     # [512]
        tmt = np.where(b == 0, Tp, 0.0).astype(np.float32)
        tmb = np.where(b == 1, Tp, 0.0).astype(np.float32)
        tm2 = np.empty((128, _NP), np.float32)
        tm2[0:64, :] = tmt[None, :]
        tm2[64:128, :] = tmb[None, :]

        in_maps.append({"mt": mt, "maskd": maskd,
                        "tm2": np.ascontiguousarray(tm2)})
    return in_maps


def kernel(x, weights_t, weights_r, r_zeros, r_const):
    from concourse.bass_utils import run_bass_kernel_spmd

    x = np.asarray(x, np.float32)
    weights_t = np.asarray(weights_t, np.float32)
    r_const = np.asarray(r_const, np.float32)
    r_zeros_np = np.asarray(r_zeros)
    if np.any(r_zeros_np):
        r_const = (np.asarray(weights_r, np.float32)
                   * r_zeros_np.astype(np.float32) + r_const)

    nc = _get_program()
    in_maps = _build_in_maps(x, weights_t, r_const)
    res = run_bass_kernel_spmd(nc, in_maps, core_ids=list(range(_NCORES)))
    # out[p, h] = sum over half-h pair-columns of vc at partition p=64b+i;
    # fold b and h here (part of the unshard/gather)
    parts = np.stack([r["out"] for r in res.results])        # [8, 128, 2]
    parts = parts.sum(axis=(0, 2), dtype=np.float64)         # [128]
    return (parts[:_N] + parts[_N:]).astype(np.float32)


# revision 32
# speedup vs baseline: 1.0270x; 1.0270x over previous
"""Trainium2 Bass kernel for nn_DegreePrediction (RBC via batched Perron vectors).

Math: M[s,t] = weights_r*r_zeros + r_const is positive column-stochastic
(columns sum to 1); its eigenvalue-1 right eigenvector is the Perron
vector and rbc[n] = sum_{s,t} T[s,t]/v[s,t,s] * v[s,t,n] is scale-free in
v.  v ~= M^2 @ ones to ~lambda2^2 ~ 0.4% << the 2e-2 gate, so two batched
mat-vec sweeps suffice (no squarings, no transposes).

Layout trick: each core's 512 matrices are uploaded TRANSPOSED in fp8-e3m4
(prescaled by 128 so entries sit in the e3m4 normal range; the result is
scale-free), two per 128-partition stack: MT[j+64h, 64q+i] ~ M_{2q+h}[i,j].
With lhsT = a [128,128] MT block (stationary operand) both sweeps keep
their results in the PARTITION dim:
  pass A: rhs = ones-blocks [128,2]       -> out[m,n] = rowsums w_p[m]
  pass B: rhs = block-diag w cols [128,4] -> out[m,n] = v_p[m]
fp8 LDWEIGHTS/MATMUL pairs pipeline through the PE reorder window at
~27ns/pair, so the kernel is paced by the 8-chunk 256KB DMA stream and
the cross-engine dependency hops.  The pipeline is split in column halves
so pass B of half 0 and its tail overlap the work of half 1.

Tail: the denominator row v_p[s_p] is gathered with a host mask (fp8,
issued as the 2nd DMA so it lands early) * VV in f32 on DVE, then an
fp32 ones-matmul partition-reduce -> d; 1/d = exp(-ln d) on ACT with both
functions forced into ONE act table (loaded once, in the DMA window);
coefficients T/d are broadcast down the partitions with a tiny PE matmul
and the final half-fold + column->row transpose is a single K=128 matmul
so the output DMA is one contiguous 256B descriptor.

Sharding: pairs split by s across 8 cores; host sums the partials.
"""

import numpy as np

_N = 64
_NCORES = 8
_NP = 512          # pairs per core
_NQ = 128          # double-stacks (4 pairs each)
_NCHUNK = 8        # DMA chunks of MT
_CW = _NQ * 128 // _NCHUNK   # MT cols per chunk (2048)
_QPC = _NQ // _NCHUNK        # double-stacks per chunk (16)
_MSCALE = 128.0    # power-of-2 prescale so fp8-e3m4 entries are normal

_cached = {}


def _build_program():
    import concourse.tile as tile
    from concourse import bacc, mybir
    from contextlib import ExitStack

    f32 = mybir.dt.float32
    bf16 = mybir.dt.bfloat16
    fp8 = mybir.dt.float8e3
    AF = mybir.ActivationFunctionType

    class _BaccOneActTable(bacc.Bacc):
        """Force Ln/Exp/Copy onto the single act-func set that contains all
        of them (natural_log_exp_and_others), so exactly one ACT_TABLE_LOAD
        is emitted (hoistable) instead of a Ln/Exp table ping-pong (3 extra
        1.5us loads in the tail).  Indices of the table list are preserved,
        only the contents of the other sets are blanked, so the
        act_func_set_id written into the BIR stays globally correct."""

        def insert_act_table_loads(self):
            import bass_rust as _bass_rust
            from concourse.hw_specs import get_activation_tables

            has_activation = any(
                isinstance(i, mybir.InstActivation)
                for b in self.main_func.blocks
                for i in b.instructions
            )
            if not has_activation:
                return
            tables = [
                (n, (f if n == "natural_log_exp_and_others" else set()))
                for n, f in get_activation_tables(self.m.arch).items()
            ]
            _bass_rust.insert_act_table_loads(self, tables)

    nc = _BaccOneActTable("TRN2", target_bir_lowering=False, debug=False)
    mt_in = nc.dram_tensor("mt", [_NCHUNK, 128, _CW], fp8,
                           kind="ExternalInput").ap()
    maskd_in = nc.dram_tensor("maskd", [128, _NP], fp8, kind="ExternalInput").ap()
    tm2_in = nc.dram_tensor("tm2", [128, _NP], f32, kind="ExternalInput").ap()
    out_dram = nc.dram_tensor("out", [128, 2], f32, kind="ExternalOutput").ap()

    with tile.TileContext(nc) as tc:
        with ExitStack() as ctx:
            consts = ctx.enter_context(tc.tile_pool(name="consts", bufs=1))
            psum = ctx.enter_context(tc.tile_pool(name="psum", bufs=1, space="PSUM"))

            # ---- stream MT chunks first (contiguous, in order); maskd is
            # the 2nd DMA so the half-0 dmm never waits on it; the other
            # small inputs trail the chunk stream ----
            mtc = []
            for d in range(_NCHUNK):
                t = consts.tile([128, _CW], fp8, tag=f"mt{d}")
                if d == 0:
                    # split the PE-gating first chunk so matmuls can start
                    # after half the transfer
                    nc.sync.dma_start(out=t[:, 0:_CW // 2],
                                      in_=mt_in[d, :, 0:_CW // 2])
                    nc.sync.dma_start(out=t[:, _CW // 2:_CW],
                                      in_=mt_in[d, :, _CW // 2:_CW])
                else:
                    nc.sync.dma_start(out=t[:, :], in_=mt_in[d, :, :])
                mtc.append(t)
                if d == 0:
                    maskd = consts.tile([128, _NP], fp8)
                    nc.sync.dma_start(out=maskd[:, :], in_=maskd_in[:, :])

            tm2 = consts.tile([128, _NP], f32)
            nc.sync.dma_start(out=tm2[:, :], in_=tm2_in[:, :])

            ones2 = consts.tile([128, 2], bf16)
            ones128 = consts.tile([128, 1], bf16)
            nc.vector.memset(ones128[:, :], 1.0)
            nc.vector.memset(ones2[:, :], 0.0)
            nc.vector.memset(ones2[0:64, 0:1], 1.0)
            nc.vector.memset(ones2[64:128, 1:2], 1.0)
            one1 = consts.tile([1, 1], f32)
            nc.vector.memset(one1[:, :], 1.0)
            onesrow = consts.tile([1, 128], bf16)
            nc.vector.memset(onesrow[:, :], 1.0)
            L = consts.tile([128, 4 * _NQ], bf16)
            nc.vector.memset(L[:, :], 0.0)
            # hoist the single ln/exp ACT table load into the DMA window
            scratch = consts.tile([1, 1], f32)
            with tc.high_priority():
                nc.scalar.activation(out=scratch[:, :], in_=one1[:, :],
                                     func=AF.Ln)

            WW = psum.tile([128, 2 * _NQ], f32, tag="WW")
            VV = psum.tile([128, _NP], f32, tag="VV")
            DB = psum.tile([128, _NP], f32, tag="DB")
            WWv = WW[:, :].rearrange("p (q two) -> p q two", two=2)
            Lv = L[:, :].rearrange("p (q four) -> p q four", four=4)
            dmm = consts.tile([128, _NP], bf16)
            U = consts.tile([128, _NP], f32)
            DPS = psum.tile([1, _NP], f32, tag="DPS")
            dinv = consts.tile([1, _NP], f32)
            dinvb = consts.tile([1, _NP], bf16)
            vc = consts.tile([128, _NP], f32)
            r1 = consts.tile([128, 2], f32)

            def sweepA(h):
                for Q in range(64 * h, 64 * h + 64):
                    d, r = Q // _QPC, Q % _QPC
                    nc.tensor.matmul(
                        out=WW[:, 2 * Q:2 * Q + 2],
                        lhsT=mtc[d][:, 128 * r:128 * r + 128],
                        rhs=ones2[:, :], start=True, stop=True)

            def lbuild(h):
                qs = slice(64 * h, 64 * h + 64)
                nc.vector.tensor_copy(out=Lv[0:64, qs, 0], in_=WWv[0:64, qs, 0])
                nc.vector.tensor_copy(out=Lv[64:128, qs, 1], in_=WWv[0:64, qs, 1])
                nc.vector.tensor_copy(out=Lv[0:64, qs, 2], in_=WWv[64:128, qs, 0])
                nc.vector.tensor_copy(out=Lv[64:128, qs, 3], in_=WWv[64:128, qs, 1])

            def sweepB(h):
                for Q in range(64 * h, 64 * h + 64):
                    d, r = Q // _QPC, Q % _QPC
                    nc.tensor.matmul(
                        out=VV[:, 4 * Q:4 * Q + 4],
                        lhsT=mtc[d][:, 128 * r:128 * r + 128],
                        rhs=L[:, 4 * Q:4 * Q + 4], start=True, stop=True)

            def tail_dve(h):
                sl = slice(256 * h, 256 * h + 256)
                nc.vector.tensor_mul(out=dmm[:, sl], in0=VV[:, sl],
                                     in1=maskd[:, sl])

            def tail_d(h):
                sl = slice(256 * h, 256 * h + 256)
                # partition-reduce via ones-matmul (fp32 to keep d exact;
                # a GpSimd axis=C reduce is ~20x slower, measured)
                nc.tensor.matmul(out=DPS[:, sl], lhsT=ones128[:, :],
                                 rhs=dmm[:, sl], start=True, stop=True)

            def tail_u(h):
                sl = slice(256 * h, 256 * h + 256)
                # off the critical chain: T-weighting of v, no d dependency
                nc.vector.tensor_mul(out=U[:, sl], in0=VV[:, sl],
                                     in1=tm2[:, sl])

            def tail_coef(h):
                sl = slice(256 * h, 256 * h + 256)
                # 1/d = exp(-ln d): d > 0 (Perron); both funcs live in the
                # single preloaded table (ACT Reciprocal is banned for
                # accuracy, DVE divide on a 1-partition row is 8 cyc/elem)
                # ln in f32 (bf16 here would exponentiate into ~4% error);
                # the single bf16 rounding happens at the exp output
                nc.scalar.activation(out=dinv[:, sl], in_=DPS[:, sl],
                                     func=AF.Ln)
                nc.scalar.activation(out=dinvb[:, sl], in_=dinv[:, sl],
                                     func=AF.Exp, scale=-1.0)

            def tail_db(h):
                sl = slice(256 * h, 256 * h + 256)
                # broadcast dinv down all partitions: outer(ones, dinv)
                nc.tensor.matmul(out=DB[:, sl], lhsT=onesrow[:, :],
                                 rhs=dinvb[:, sl], start=True, stop=True)

            def tail_fin(h):
                sl = slice(256 * h, 256 * h + 256)
                nc.vector.tensor_mul(out=vc[:, sl], in0=U[:, sl],
                                     in1=DB[:, sl])
                nc.vector.tensor_reduce(
                    out=r1[:, h:h + 1], in_=vc[:, sl],
                    axis=mybir.AxisListType.X, op=mybir.AluOpType.add)

            # PE queue: A0 A1 B0 DPS0 B1 CB0 DPS1 CB1 FR.  Both A-sweeps
            # run first (purely DMA-paced; the lbuild casts overlap them on
            # DVE), so B0/B1 run back-to-back right after the last chunk
            # lands and the serial half-1 tail chain starts ~5us earlier,
            # with half-0's chain overlapping it on ACT/DVE.
            sweepA(0)
            lbuild(0)
            sweepA(1)
            lbuild(1)
            sweepB(0)
            tail_dve(0)
            tail_u(0)
            tail_d(0)
            sweepB(1)
            tail_coef(0)
            tail_dve(1)    # dmm(1)/U(1) precede fin(0) in the DVE queue:
            tail_u(1)      # they head the serial half-1 chain
            tail_db(0)
            tail_d(1)
            tail_coef(1)
            tail_fin(0)
            tail_db(1)
            tail_fin(1)

            # ship both half-reduces raw; the b-fold and node transpose
            # happen on host inside the existing cross-core gather
            nc.sync.dma_start(out=out_dram[:, :], in_=r1[:, :])
    nc.compile()
    return nc


def _get_program():
    if "nc" not in _cached:
        _cached["nc"] = _build_program()
    return _cached["nc"]


def _build_in_maps(x, weights_t, r_const):
    """Host-side layouts for all 8 cores."""
    import ml_dtypes

    M_all = r_const.reshape(_N * _N, _N, _N)
    i = np.arange(_N)
    r_diag = r_const[i[:, None], i[None, :], i[:, None], i[:, None]]
    T_full = (x * weights_t * r_diag).astype(np.float32)      # [64, 64]

    p = np.arange(_NP)
    b = (p >> 1) & 1                                          # stack-half of pair
    s_loc = p >> 6
    t_loc = p & 63

    in_maps = []
    for c in range(_NCORES):
        Mc = np.asarray(M_all[_NP * c:_NP * (c + 1)], np.float32)  # (p,i,j)
        # MT[j+64h, 64(2Q+b)+i] = scale*Mc[4Q+2b+h, i, j], chunked contiguously
        mt = ((Mc * _MSCALE).reshape(_NQ, 2, 2, _N, _N)  # (Q, b, h, i, j)
              .transpose(2, 4, 0, 1, 3)                  # (h, j, Q, b, i)
              .reshape(128, _NQ * 128))
        mt = (mt.reshape(128, _NCHUNK, _CW).transpose(1, 0, 2))  # (chunk, p, f)
        mt = np.ascontiguousarray(mt).astype(ml_dtypes.float8_e3m4)

        maskd = np.zeros((128, _NP), np.float32)
        maskd[64 * b + 8 * c + s_loc, p] = 1.0   # v's node index is GLOBAL s
        maskd = maskd.astype(ml_dtypes.float8_e3m4)

        Tp = T_full[8 * c + s_loc, t_loc]                     # [512]
        tmt = np.where(b == 0, Tp, 0.0).astype(np.float32)
        tmb = np.where(b == 1, Tp, 0.0).astype(np.float32)
        tm2 = np.empty((128, _NP), np.float32)
        tm2[0:64, :] = tmt[None, :]
        tm2[64:128, :] = tmb[None, :]

        in_maps.append({"mt": mt, "maskd": maskd,
                        "tm2": np.ascontiguousarray(tm2)})
    return in_maps


def kernel(x, weights_t, weights_r, r_zeros, r_const):
    from concourse.bass_utils import run_bass_kernel_spmd

    x = np.asarray(x, np.float32)
    weights_t = np.asarray(weights_t, np.float32)
    r_const = np.asarray(r_const, np.float32)
    r_zeros_np = np.asarray(r_zeros)
    if np.any(r_zeros_np):
        r_const = (np.asarray(weights_r, np.float32)
                   * r_zeros_np.astype(np.float32) + r_const)

    nc = _get_program()
    in_maps = _build_in_maps(x, weights_t, r_const)
    res = run_bass_kernel_spmd(nc, in_maps, core_ids=list(range(_NCORES)))
    # out[p, h] = sum over half-h pair-columns of vc at partition p=64b+i;
    # fold b and h here (part of the unshard/gather)
    parts = np.stack([r["out"] for r in res.results])        # [8, 128, 2]
    parts = parts.sum(axis=(0, 2), dtype=np.float64)         # [128]
    return (parts[:_N] + parts[_N:]).astype(np.float32)
